# revision 6
# baseline (speedup 1.0000x reference)
"""Trainium2 Bass kernel for a rate-1/2, constraint-length-3 feedforward
convolutional encoder (generator polynomials "101" and "111", MSB-first).

The trellis scan in the reference collapses to elementwise XORs of shifted
input bits (zero initial state):

    out0[t] = u[t] ^ u[t-2]            (poly "101")
    out1[t] = u[t] ^ u[t-1] ^ u[t-2]   (poly "111")

with the codeword interleaved time-major: y[:, 2t] = out0[t], y[:, 2t+1] = out1[t].

XOR on {0,1} floats is computed arithmetically: x ^ y = (x - y)^2.

Sharding: pure data parallel over the batch dim across 8 NeuronCores.
The kernel is DMA-bound (3 MiB of HBM traffic per 1 MiB of input); the
compute (2 vector + 2 scalar ops per tile) hides entirely under the DMA.
"""

import numpy as np

N_CORES = 8
B, K = 8192, 2048
N_OUT = 2
SHARD_B = B // N_CORES  # 1024 codewords per core
P = 128                 # SBUF partitions
HK = K // 2             # column half processed per iteration

_compiled = {}


def _build_nc():
    import concourse.bass as bass  # noqa: F401
    import concourse.tile as tile
    from concourse import bacc, mybir

    nc = bacc.Bacc(
        "TRN2",
        target_bir_lowering=False,
        debug=False,
        enable_asserts=False,
    )
    x = nc.dram_tensor("x", [SHARD_B, K], mybir.dt.float32, kind="ExternalInput").ap()
    y = nc.dram_tensor(
        "y", [SHARD_B, N_OUT * K], mybir.dt.float32, kind="ExternalOutput"
    ).ap()

    n_groups = SHARD_B // P  # 8 row-groups of 128
    N_SLOTS = 6

    with tile.TileContext(nc) as tc:
        with (
            tc.tile_pool(name="xin", bufs=1) as in_pool,
            tc.tile_pool(name="out", bufs=5) as out_pool,
            tc.tile_pool(name="tmp", bufs=4) as tmp_pool,
        ):
            # Persistent input slots with 2 leading zero columns so the
            # shifted views u[t-1], u[t-2] fall out of plain column offsets.
            # The zero columns are written ONCE here; the per-iteration DMAs
            # only write cols [2:], so no DMA ever waits on a memset.
            in_slots = [
                in_pool.tile(
                    [P, K + 2], mybir.dt.float32, tag=f"xin{j}", name=f"xin{j}"
                )
                for j in range(N_SLOTS)
            ]
            for j in range(N_SLOTS):
                nc.vector.memset(in_slots[j][:, 0:2], 0.0)

            # Column-halved iterations: smaller first transfers start the
            # output stream earlier, keeping the DMA engines saturated with
            # mixed read+write traffic for most of the kernel.
            for g in range(n_groups):
                xin = in_slots[g % N_SLOTS]
                rows = slice(g * P, (g + 1) * P)
                for h in range(2):
                    lo = h * HK
                    # Input DMAs on the SP HWDGE ring (Sync sequencer).
                    nc.sync.dma_start(
                        xin[:, 2 + lo : 2 + lo + HK], x[rows, lo : lo + HK]
                    )

                    a = xin[:, 2 + lo : 2 + lo + HK]  # u[t]
                    b = xin[:, 1 + lo : 1 + lo + HK]  # u[t-1]
                    c = xin[:, lo : lo + HK]          # u[t-2]

                    out = out_pool.tile(
                        [P, N_OUT * HK], mybir.dt.float32, tag="out", name="out"
                    )
                    even = out[:, 0 : N_OUT * HK : 2]
                    odd = out[:, 1 : N_OUT * HK : 2]

                    # p = a - c in {-1,0,1}; out0 = p^2 = a ^ c
                    p = tmp_pool.tile([P, HK], mybir.dt.float32, tag="p", name="p")
                    nc.vector.tensor_tensor(p[:], a, c, mybir.AluOpType.subtract)
                    nc.scalar.square(even, p[:])

                    # q = out0 - b in {-1,0,1}; out1 = q^2 = out0 ^ b
                    # (reuses p's buffer: p is dead once the first square ran)
                    nc.vector.tensor_tensor(p[:], even, b, mybir.AluOpType.subtract)
                    nc.scalar.square(odd, p[:])

                    # Output DMAs on the SWDGE path (GpSimd sequencer) so a
                    # stalled input-DMA trigger never blocks a ready output
                    # DMA (and vice versa) — the streams issue independently.
                    nc.gpsimd.dma_start(
                        y[rows, N_OUT * lo : N_OUT * (lo + HK)], out[:]
                    )

    nc.compile()
    return nc


def _get_nc():
    if "nc" not in _compiled:
        _compiled["nc"] = _build_nc()
    return _compiled["nc"]


def kernel(**inputs) -> np.ndarray:
    from concourse.bass_utils import run_bass_kernel_spmd

    x_full = np.ascontiguousarray(np.asarray(inputs["inputs"], dtype=np.float32))
    assert x_full.shape == (B, K), x_full.shape

    nc = _get_nc()
    in_maps = [
        {"x": x_full[i * SHARD_B : (i + 1) * SHARD_B]} for i in range(N_CORES)
    ]
    res = run_bass_kernel_spmd(nc, in_maps, core_ids=list(range(N_CORES)))
    out = np.concatenate([r["y"] for r in res.results], axis=0)
    return np.ascontiguousarray(out, dtype=np.float32)


# revision 7
# speedup vs baseline: 1.1208x; 1.1208x over previous
"""Trainium2 Bass kernel for a rate-1/2, constraint-length-3 feedforward
convolutional encoder (generator polynomials "101" and "111", MSB-first).

The trellis scan in the reference collapses to elementwise XORs of shifted
input bits (zero initial state):

    out0[t] = u[t] ^ u[t-2]            (poly "101")
    out1[t] = u[t] ^ u[t-1] ^ u[t-2]   (poly "111")

with the codeword interleaved time-major: y[:, 2t] = out0[t], y[:, 2t+1] = out1[t].

XOR on {0,1} floats is computed arithmetically: x ^ y = (x - y)^2.

Sharding: pure data parallel over the batch dim across 8 NeuronCores.
The kernel is DMA-bound (3 MiB of HBM traffic per 1 MiB of input); the
compute (2 vector + 2 scalar ops per tile) hides entirely under the DMA.
"""

import numpy as np

N_CORES = 8
B, K = 8192, 2048
N_OUT = 2
SHARD_B = B // N_CORES  # 1024 codewords per core
P = 128                 # SBUF partitions
HK = K // 2             # column half processed per iteration

_compiled = {}


def _build_nc():
    import concourse.bass as bass  # noqa: F401
    import concourse.tile as tile
    from concourse import bacc, mybir

    nc = bacc.Bacc(
        "TRN2",
        target_bir_lowering=False,
        debug=False,
        enable_asserts=False,
    )
    x = nc.dram_tensor("x", [SHARD_B, K], mybir.dt.float32, kind="ExternalInput").ap()
    y = nc.dram_tensor(
        "y", [SHARD_B, N_OUT * K], mybir.dt.float32, kind="ExternalOutput"
    ).ap()

    n_groups = SHARD_B // P  # 8 row-groups of 128
    N_SLOTS = 6

    with tile.TileContext(nc) as tc:
        with (
            tc.tile_pool(name="xin", bufs=1) as in_pool,
            tc.tile_pool(name="out", bufs=5) as out_pool,
            tc.tile_pool(name="tmp", bufs=4) as tmp_pool,
        ):
            # Persistent input slots with 2 leading zero columns so the
            # shifted views u[t-1], u[t-2] fall out of plain column offsets.
            # The zero columns are written ONCE here; the per-iteration DMAs
            # only write cols [2:], so no DMA ever waits on a memset.
            in_slots = [
                in_pool.tile(
                    [P, K + 2], mybir.dt.float32, tag=f"xin{j}", name=f"xin{j}"
                )
                for j in range(N_SLOTS)
            ]
            for j in range(N_SLOTS):
                nc.vector.memset(in_slots[j][:, 0:2], 0.0)

            for g in range(n_groups):
                xin = in_slots[g % N_SLOTS]
                rows = slice(g * P, (g + 1) * P)
                # Input DMAs on the SP HWDGE ring (Sync sequencer).
                nc.sync.dma_start(xin[:, 2 : 2 + K], x[rows, :])

                a = xin[:, 2 : 2 + K]  # u[t]
                b = xin[:, 1 : 1 + K]  # u[t-1]
                c = xin[:, 0:K]        # u[t-2]

                out = out_pool.tile(
                    [P, N_OUT * K], mybir.dt.float32, tag="out", name="out"
                )
                even = out[:, 0 : N_OUT * K : 2]
                odd = out[:, 1 : N_OUT * K : 2]

                # p = a - c in {-1,0,1}; out0 = p^2 = a ^ c
                p = tmp_pool.tile([P, K], mybir.dt.float32, tag="p", name="p")
                nc.vector.tensor_tensor(p[:], a, c, mybir.AluOpType.subtract)
                nc.scalar.square(even, p[:])

                # q = out0 - b in {-1,0,1}; out1 = q^2 = out0 ^ b
                # (reuses p's buffer: p is dead once the first square ran)
                nc.vector.tensor_tensor(p[:], even, b, mybir.AluOpType.subtract)
                nc.scalar.square(odd, p[:])

                # Output DMAs on the ACT HWDGE ring (Scalar sequencer): a
                # separate ring from the input DMAs so neither stream's
                # stalled trigger blocks the other, and HWDGE avoids the
                # SWDGE descriptor rings in SBUF that contend with data
                # ports. The trigger lands right after the square that
                # finishes the tile, so it never stalls the ACT sequencer.
                nc.scalar.dma_start(y[rows, :], out[:])

    nc.compile()
    return nc


def _get_nc():
    if "nc" not in _compiled:
        _compiled["nc"] = _build_nc()
    return _compiled["nc"]


def kernel(**inputs) -> np.ndarray:
    from concourse.bass_utils import run_bass_kernel_spmd

    x_full = np.ascontiguousarray(np.asarray(inputs["inputs"], dtype=np.float32))
    assert x_full.shape == (B, K), x_full.shape

    nc = _get_nc()
    in_maps = [
        {"x": x_full[i * SHARD_B : (i + 1) * SHARD_B]} for i in range(N_CORES)
    ]
    res = run_bass_kernel_spmd(nc, in_maps, core_ids=list(range(N_CORES)))
    out = np.concatenate([r["y"] for r in res.results], axis=0)
    return np.ascontiguousarray(out, dtype=np.float32)
